# revision 8
# baseline (speedup 1.0000x reference)
"""Trainium2 Bass kernel for nn_CMHAttention (Linformer-style attention).

Sharding: 8 cores; core c owns sequence rows [c*512, (c+1)*512) of every batch.
Each core computes Q/K/V projections for its rows, partial E/F sequence
projections (Kp/Vp) over its s-chunk, one 8-rank AllReduce combines the
partials, then each core finishes attention + output projection for its rows.

Compute dtype: bf16 matmuls with fp32 PSUM accumulation (rel err ~7e-3 vs
fp32 reference, validated against a numpy prototype of this exact blocking).
"""

import functools

import ml_dtypes
import numpy as np

import concourse.bacc as bacc
import concourse.tile as tile
from concourse import mybir
from concourse.bass_utils import run_bass_kernel_spmd

BF16 = ml_dtypes.bfloat16

B, S, C = 4, 4096, 1024
H, D, K = 16, 64, 256
NCORES = 8
SC = S // NCORES          # 512 sequence rows per core per batch
R = B * SC                # 2048 rows per core (row r = b*SC + s_local)
HD = H * D                # 1024
CT = C // 128             # 8 c-tiles
ST = R // 128             # 16 row-tiles
SQ = SC // 128            # 4 s-subtiles per batch
KSUB = K // 128           # 2 k-subtiles
BH_ELEMS = D * K          # 16384 elements per (b,h) slot in the AR buffer

bf = mybir.dt.bfloat16
f32 = mybir.dt.float32
f32r = mybir.dt.float32r


@functools.lru_cache(maxsize=1)
def _build():
    nc = bacc.Bacc("TRN2", target_bir_lowering=False, debug=False,
                   num_devices=NCORES)

    xb = nc.dram_tensor("xb", [R, C], bf, kind="ExternalInput")
    wqT = nc.dram_tensor("wqT", [C, HD], bf, kind="ExternalInput")
    wkT = nc.dram_tensor("wkT", [C, HD], bf, kind="ExternalInput")
    wvT = nc.dram_tensor("wvT", [C, HD], bf, kind="ExternalInput")
    weT = nc.dram_tensor("weT", [SC, H, K], bf, kind="ExternalInput")
    wfT = nc.dram_tensor("wfT", [SC, H, K], bf, kind="ExternalInput")
    woT = nc.dram_tensor("woT", [HD, C], bf, kind="ExternalInput")
    bo_d = nc.dram_tensor("bo", [1, C], f32, kind="ExternalInput")
    out_d = nc.dram_tensor("out", [R, C], f32, kind="ExternalOutput")

    # AllReduce bounce buffers: [2 (kp|vp), B, H, D*K] fp32.
    # kp slot (b,h): row-major [d, k]; vp slot (b,h): row-major [k, d].
    cc_in = nc.dram_tensor("cc_in", [2, B, H, BH_ELEMS], f32)
    cc_out = nc.dram_tensor("cc_out", [2, B, H, BH_ELEMS], f32,
                            addr_space="Shared")

    with tile.TileContext(nc) as tc:
        p_const = tc.alloc_tile_pool(name="const", bufs=1)
        ps = tc.alloc_tile_pool(name="ps", bufs=6, space="PSUM")

        # ---- constants ----
        ones_f = p_const.tile([1, 64], f32, tag="onesf")
        nc.vector.memset(ones_f[:, :], 1.0)
        ones_r = p_const.tile([1, 64], f32r, tag="onesr")
        nc.vector.tensor_copy(ones_r[:, :], ones_f[:, :])
        bo_bc = p_const.tile([128, C], f32, tag="bo")
        nc.sync.dma_start(out=bo_bc[:, :], in_=bo_d[0, :].partition_broadcast(128))

        # ---- phase pools (released in LIFO order) ----
        p_ctx = tc.alloc_tile_pool(name="ctx", bufs=1)
        ctxT = [p_ctx.tile([128, R], bf, tag=f"ctx{i}", name=f"ctx{i}")
                for i in range(CT)]
        p_xt = tc.alloc_tile_pool(name="xt", bufs=1)
        p_w = tc.alloc_tile_pool(name="w", bufs=2)
        p_kv = tc.alloc_tile_pool(name="kv", bufs=1)
        p_wef = tc.alloc_tile_pool(name="wef", bufs=3)
        p_stg = tc.alloc_tile_pool(name="stg", bufs=6)

        # ---- xT: transpose-load x [R, C] -> 8 tiles [128 c, R] ----
        xT = []
        for ct in range(CT):
            t = p_xt.tile([128, R], bf, tag=f"xt{ct}", name=f"xt{ct}")
            nc.sync.dma_start(out=t[:, :], in_=xb[:, ct * 128:(ct + 1) * 128],
                              transpose=True)
            xT.append(t)

        def load_w(dram, nm):
            # [C, HD] -> sbuf [128 c-part, CT, HD]
            t = p_w.tile([128, CT, HD], bf, tag="w", name=nm)
            nc.sync.dma_start(
                out=t[:, :, :],
                in_=dram.ap().rearrange("(ct p) hd -> p ct hd", p=128))
            return t

        # ---- K, V projections: natural [row, hd] ----
        def proj_rows(w_sb, nm):
            tiles = []
            for st in range(ST):
                t = p_kv.tile([128, HD], bf, tag=f"{nm}{st}", name=f"{nm}{st}")
                for n in range(2):
                    pt = ps.tile([128, 512], f32, tag="mm", name="pmm")
                    for ct in range(CT):
                        nc.tensor.matmul(
                            pt[:, :],
                            xT[ct][:, st * 128:(st + 1) * 128],
                            w_sb[:, ct, n * 512:(n + 1) * 512],
                            start=(ct == 0), stop=(ct == CT - 1))
                    nc.vector.tensor_copy(t[:, n * 512:(n + 1) * 512], pt[:, :])
                tiles.append(t)
            return tiles

        wk_sb = load_w(wkT, "wk")
        K_sb = proj_rows(wk_sb, "k")
        wv_sb = load_w(wvT, "wv")
        V_sb = proj_rows(wv_sb, "v")

        # ---- Kp/Vp partials, head-major so We/Wf tiles stream ----
        for h in range(H):
            we_h = p_wef.tile([128, SQ, K], bf, tag="we", name="we")
            nc.sync.dma_start(
                out=we_h[:, :, :],
                in_=weT.ap()[:, h, :].rearrange("(sq p) k -> p sq k", p=128))
            wf_h = p_wef.tile([128, SQ, K], bf, tag="wf", name="wf")
            nc.sync.dma_start(
                out=wf_h[:, :, :],
                in_=wfT.ap()[:, h, :].rearrange("(sq p) k -> p sq k", p=128))

            # Kp: psum [64 d, 256 k] per (b, h)
            for b in range(B):
                pt = ps.tile([64, K], f32, tag="mm", name="pkp")
                for sq in range(SQ):
                    nc.tensor.matmul(
                        pt[:, :],
                        K_sb[SQ * b + sq][:, h * D:(h + 1) * D],
                        we_h[:, sq, :],
                        start=(sq == 0), stop=(sq == SQ - 1))
                stg = p_stg.tile([64, K], f32, tag="kstg", name="kstg")
                nc.vector.tensor_copy(stg[:, :], pt[:, :])
                nc.sync.dma_start(
                    out=cc_in.ap()[0, b, h, :].rearrange("(d k) -> d k", k=K),
                    in_=stg[:, :])

            # Vp: psum [128 k, 64 d] per (h, ksub, b); same lhsT reused over b
            for ksub in range(KSUB):
                pts = [ps.tile([128, D], f32, tag="mm", name=f"pvp{b}")
                       for b in range(B)]
                for sq in range(SQ):
                    for b in range(B):
                        nc.tensor.matmul(
                            pts[b][:, :],
                            wf_h[:, sq, ksub * 128:(ksub + 1) * 128],
                            V_sb[SQ * b + sq][:, h * D:(h + 1) * D],
                            start=(sq == 0), stop=(sq == SQ - 1))
                stg = p_stg.tile([128, B, D], f32, tag="vstg", name="vstg")
                for b in range(B):
                    nc.vector.tensor_copy(stg[:, b, :], pts[b][:, :])
                # cc vp slot (b,h): addr k*D + d ; k = ksub*128 + p
                nc.sync.dma_start(
                    out=cc_in.ap()[1, :, h, :]
                    .rearrange("b (k2 p d) -> p k2 b d", p=128, d=D)[:, ksub, :, :],
                    in_=stg[:, :, :])

        # ---- AllReduce of Kp/Vp partials across all 8 cores ----
        nc.gpsimd.collective_compute(
            "AllReduce", mybir.AluOpType.add,
            replica_groups=[list(range(NCORES))],
            ins=[cc_in[:, :, :, :]],
            outs=[cc_out[:, :, :, :]],
        )

        p_stg.release()
        p_wef.release()
        p_kv.release()

        # ---- Q projection (overlaps the AllReduce): QT [hd, row] ----
        p_qt = tc.alloc_tile_pool(name="qt", bufs=1)
        wq_sb = load_w(wqT, "wq")
        QT = []
        for ht in range(CT):
            t = p_qt.tile([128, R], bf, tag=f"qt{ht}", name=f"qt{ht}")
            for n in range(R // 512):
                pt = ps.tile([128, 512], f32, tag="mm", name="pq")
                for ct in range(CT):
                    nc.tensor.matmul(
                        pt[:, :],
                        wq_sb[:, ct, ht * 128:(ht + 1) * 128],
                        xT[ct][:, n * 512:(n + 1) * 512],
                        start=(ct == 0), stop=(ct == CT - 1))
                nc.vector.tensor_copy(t[:, n * 512:(n + 1) * 512], pt[:, :])
            QT.append(t)

        # ---- load back reduced Kp/Vp as bf16 (casting SWDGE DMA) ----
        p_big = tc.alloc_tile_pool(name="big", bufs=1)
        # kp_bf: [128 p=(h%2)*64+d, hp, b, k]
        kp_bf = p_big.tile([128, H // 2, B, K], bf, tag="kpbf", name="kpbf")
        for b in range(B):
            nc.gpsimd.dma_start(
                out=kp_bf[:, :, b, :],
                in_=cc_out.ap()[0, b, :, :]
                .rearrange("h (d k) -> (h d) k", k=K)
                .rearrange("(hp p) k -> p hp k", p=128))
        # vp_bf: [128 p=k%128, ksub, b, h, 65] with a trailing ones column
        vp_bf = p_big.tile([128, KSUB, B, H, D + 1], bf, tag="vpbf", name="vpbf")
        for b in range(B):
            for ksub in range(KSUB):
                nc.gpsimd.dma_start(
                    out=vp_bf[:, ksub, b, :, 0:D],
                    in_=cc_out.ap()[1, b, :, :]
                    .rearrange("h (k2 p d) -> p k2 h d", p=128, d=D)[:, ksub, :, :])
        nc.vector.memset(vp_bf[:, :, :, :, D:D + 1], 1.0)

        # ---- attention per (b, h) ----
        p_e = tc.alloc_tile_pool(name="e", bufs=8)
        p_rc = tc.alloc_tile_pool(name="rc", bufs=2)
        for b in range(B):
            for h in range(H):
                hp, hl = h // 2, (h % 2) * 64
                e_t = []
                for ksub in range(KSUB):
                    pst = ps.tile([128, 512], f32, tag="mm", name="pst")
                    nc.tensor.matmul(
                        pst[:, :],
                        kp_bf[hl:hl + 64, hp, b, ksub * 128:(ksub + 1) * 128],
                        QT[hp][hl:hl + 64, b * SC:(b + 1) * SC],
                        start=True, stop=True)
                    et = p_e.tile([128, 512], bf, tag="e", name="e")
                    nc.scalar.activation(out=et[:, :], in_=pst[:, :],
                                         func=mybir.ActivationFunctionType.Exp,
                                         scale=0.125)
                    e_t.append(et)
                # ctx+denominator: psum [65, 512]; row 64 = sum_k E
                pcd = ps.tile([D + 1, 512], f32, tag="mm", name="pcd")
                for ksub in range(KSUB):
                    nc.tensor.matmul(
                        pcd[:, :],
                        vp_bf[:, ksub, b, h, :],
                        e_t[ksub][:, :],
                        start=(ksub == 0), stop=(ksub == KSUB - 1))
                rc = p_rc.tile([1, 512], f32, tag="rc", name="rc")
                nc.vector.reciprocal(rc[:, :], pcd[D:D + 1, :])
                rcr = p_rc.tile([1, 512], f32r, tag="rcr", name="rcr")
                nc.vector.tensor_copy(rcr[:, :], rc[:, :])
                prb = ps.tile([64, 512], f32, tag="mm", name="prb")
                nc.tensor.matmul(prb[:, :], ones_r[:, :], rcr[:, :],
                                 start=True, stop=True)
                rb_sb = p_rc.tile([64, 512], f32, tag="rbsb", name="rbsb")
                nc.vector.tensor_copy(rb_sb[:, :], prb[:, :])
                nc.vector.tensor_mul(
                    ctxT[hp][hl:hl + 64, b * SC:(b + 1) * SC],
                    pcd[0:D, :], rb_sb[:, :])

        p_rc.release()
        p_e.release()
        p_big.release()
        p_qt.release()
        p_w.release()
        p_xt.release()

        # ---- output projection + bias ----
        p_wo = tc.alloc_tile_pool(name="wo", bufs=1)
        p_ob = tc.alloc_tile_pool(name="ob", bufs=3)
        wo_sb = p_wo.tile([128, CT, C], bf, tag="wo", name="wo")
        nc.sync.dma_start(
            out=wo_sb[:, :, :],
            in_=woT.ap().rearrange("(ht p) c -> p ht c", p=128))
        for st in range(ST):
            ot = p_ob.tile([128, C], f32, tag="ob", name="ob")
            for n in range(2):
                pt = ps.tile([128, 512], f32, tag="mm", name="po")
                for ht in range(CT):
                    nc.tensor.matmul(
                        pt[:, :],
                        ctxT[ht][:, st * 128:(st + 1) * 128],
                        wo_sb[:, ht, n * 512:(n + 1) * 512],
                        start=(ht == 0), stop=(ht == CT - 1))
                nc.vector.tensor_add(ot[:, n * 512:(n + 1) * 512], pt[:, :],
                                     bo_bc[:, n * 512:(n + 1) * 512])
            nc.sync.dma_start(out=out_d[st * 128:(st + 1) * 128, :],
                              in_=ot[:, :])

        p_ob.release()
        p_wo.release()
        p_ctx.release()
        p_xt2 = None  # placeholder, nothing else to release here
        ps.release()
        p_const.release()

    nc.finalize()
    return nc


def _make_in_maps(inputs):
    x = np.asarray(inputs["x"], dtype=np.float32)
    We = np.asarray(inputs["We"], np.float32)
    Wf = np.asarray(inputs["Wf"], np.float32)
    wqT = np.ascontiguousarray(
        np.asarray(inputs["Wq"], np.float32).reshape(HD, C).T).astype(BF16)
    wkT = np.ascontiguousarray(
        np.asarray(inputs["Wk"], np.float32).reshape(HD, C).T).astype(BF16)
    wvT = np.ascontiguousarray(
        np.asarray(inputs["Wv"], np.float32).reshape(HD, C).T).astype(BF16)
    woT = np.ascontiguousarray(np.asarray(inputs["Wo"], np.float32).T).astype(BF16)
    bo_h = np.asarray(inputs["bo"], np.float32).reshape(1, C)

    in_maps = []
    for c in range(NCORES):
        sc = slice(c * SC, (c + 1) * SC)
        in_maps.append({
            "xb": np.ascontiguousarray(x[:, sc, :].reshape(R, C)).astype(BF16),
            "wqT": wqT, "wkT": wkT, "wvT": wvT,
            "weT": np.ascontiguousarray(We[:, :, sc].transpose(2, 0, 1)).astype(BF16),
            "wfT": np.ascontiguousarray(Wf[:, :, sc].transpose(2, 0, 1)).astype(BF16),
            "woT": woT, "bo": bo_h,
        })
    return in_maps


def kernel(x, Wq, Wk, Wv, We, Wf, Wo, bo):
    nc = _build()
    in_maps = _make_in_maps(dict(x=x, Wq=Wq, Wk=Wk, Wv=Wv, We=We, Wf=Wf,
                                 Wo=Wo, bo=bo))
    res = run_bass_kernel_spmd(nc, in_maps, list(range(NCORES)))

    out = np.empty((B, S, C), np.float32)
    for c in range(NCORES):
        out[:, c * SC:(c + 1) * SC, :] = res.results[c]["out"].reshape(B, SC, C)
    return out
